# revision 1
# baseline (speedup 1.0000x reference)
"""Distributed Trainium2 kernel for the Koopman-operator problem.

Math (from the reference):
    X  = x.reshape(64, T)                 # T = 524288, pure row-major view
    M  = L @ L.T                          # 128x128;  M11, M21, M22 are 64x64 blocks
    B  = 2*(M11 + M22 + R - R.T)          # (eps*I is ~1e-8, negligible vs O(30) entries)
    A  = inv(B) @ M21
    out = (A @ X).reshape(-1, 64)

Distribution: column-shard X across 8 cores (65536 cols each) -- fully
data-parallel, zero collectives.  L and R are replicated; every core
redundantly computes A on device via a Frobenius-scaled Newton-Schulz
iteration (B is well conditioned, cond ~650 -> ~24 iterations converge to
the f32 floor).

Per core the shard is pre-stacked on host as (128, 32768): rows 0:64 hold
the first 32768 columns, rows 64:128 the next 32768.  The stationary
matrix for the big matmul is the block-diagonal [[A^T, 0], [0, A^T]]
(128x128), which doubles PE utilization (K=128, M=128 instead of 64).
The streaming matmul runs in float32r (full-fp32 replication mode,
1 cycle/row at N=512).
"""

import os
import sys

import numpy as np

for _p in ("/opt/trn_rl_repo", "/root/.axon_site/_ro/trn_rl_repo"):
    if _p not in sys.path and os.path.isdir(_p):
        sys.path.append(_p)

import concourse.bass as bass
import concourse.mybir as mybir
from concourse import bacc
from concourse.bass_utils import run_bass_kernel_spmd

from concourse.tile import TileContext

F32 = mybir.dt.float32
F32R = mybir.dt.float32r

N = 64                   # state dim
N_CORES = 8
T_FULL = 524288          # columns of the reshaped X
T_CORE = T_FULL // N_CORES       # 65536 columns per core
T_HALF = T_CORE // 2             # 32768 -> free dim of the (128, .) shard

N_SQUARE = 21            # number of product factors (I + E^(2^j)), j=0..20
N_POLISH = 1             # self-correcting Newton polish iterations
MM_COLS = 512            # matmul moving free dim (one PSUM bank, f32)
DMA_COLS = 4096          # input DMA chunk = 128 x 4096 x 4B = 2 MiB
OUT_COLS = 4096          # output DMA chunk = 2 MiB (fewer, bigger writes)


def build_kernel(t_half=T_HALF):
    nc = bacc.Bacc()

    x_ext = nc.declare_dram_parameter("x", [128, t_half], F32R, isOutput=False)
    lt_ext = nc.declare_dram_parameter("Lt", [2 * N, 2 * N], F32, isOutput=False)
    r_ext = nc.declare_dram_parameter("R", [N, N], F32, isOutput=False)
    rt_ext = nc.declare_dram_parameter("Rt", [N, N], F32, isOutput=False)
    twoi_ext = nc.declare_dram_parameter("TWOI", [N, N], F32, isOutput=False)
    eye_ext = nc.declare_dram_parameter("EYE", [N, N], F32, isOutput=False)
    zero_ext = nc.declare_dram_parameter("ZERO128", [128, 128], F32R, isOutput=False)
    out_ext = nc.declare_dram_parameter("out", [128, t_half], F32, isOutput=True)

    n_chunks = t_half // DMA_COLS
    mm_per_chunk = DMA_COLS // MM_COLS

    with TileContext(nc) as tc:
        with (
            tc.tile_pool(name="const", bufs=1) as cpool,
            tc.tile_pool(name="small", bufs=2) as spool,
            tc.tile_pool(name="fch", bufs=1) as fpool,
            tc.tile_pool(name="xstate", bufs=2) as xpool_s,
            tc.tile_pool(name="xin", bufs=8) as xpool,
            tc.tile_pool(name="yout", bufs=3) as opool,
            tc.tile_pool(name="pro_ps", bufs=3, space="PSUM") as pps,
            tc.tile_pool(name="nw_ps", bufs=1, space="PSUM") as nps,
            tc.tile_pool(name="mm_ps", bufs=4, space="PSUM") as mps,
        ):
            # ---- constants (DVE memsets; no gpsimd anywhere -> fewer sems) ----
            ones64 = cpool.tile([N, N], F32)
            nc.vector.memset(ones64[:], 1.0)

            # ---- load params (Lt/Rt are host-side layout of replicated L/R) ----
            lt_sb = spool.tile([128, 128], F32)
            nc.sync.dma_start(out=lt_sb[:], in_=lt_ext[:, :])
            r_sb = spool.tile([N, N], F32)
            nc.sync.dma_start(out=r_sb[:], in_=r_ext[:, :])
            rt_sb = spool.tile([N, N], F32)
            nc.sync.dma_start(out=rt_sb[:], in_=rt_ext[:, :])
            two_i = cpool.tile([N, N], F32)
            nc.sync.dma_start(out=two_i[:], in_=twoi_ext[:, :])
            eye = cpool.tile([N, N], F32)
            nc.sync.dma_start(out=eye[:], in_=eye_ext[:, :])

            # ---- S = M11 + M22 = L1@L1^T + L2@L2^T  (PSUM accumulation) ----
            s_ps = pps.tile([N, N], F32, tag="pro")
            nc.tensor.matmul(
                s_ps[:], lhsT=lt_sb[:, 0:N], rhs=lt_sb[:, 0:N], start=True, stop=False
            )
            nc.tensor.matmul(
                s_ps[:], lhsT=lt_sb[:, N:128], rhs=lt_sb[:, N:128],
                start=False, stop=True,
            )

            # ---- M21h = 0.5 * M21 = 0.5 * L1 @ L2^T ----
            # (the 0.5 folds away the factor 2 in B: inv(2*Bh) @ M21 = inv(Bh) @ (M21/2))
            m21_ps = pps.tile([N, N], F32, tag="pro")
            nc.tensor.matmul(
                m21_ps[:], lhsT=lt_sb[:, 0:N], rhs=lt_sb[:, N:128],
                start=True, stop=True,
            )
            m21h_sb = spool.tile([N, N], F32)
            nc.vector.tensor_scalar_mul(m21h_sb[:], m21_ps[:], 0.5)

            # ---- skew = R - R^T ----
            skew_sb = spool.tile([N, N], F32)
            nc.vector.tensor_sub(out=skew_sb[:], in0=r_sb[:], in1=rt_sb[:])

            # ---- Bh = S + skew (= B/2), Bth = S - skew (= B^T/2) ----
            bh_sb = spool.tile([N, N], F32)
            nc.vector.tensor_add(out=bh_sb[:], in0=s_ps[:], in1=skew_sb[:])
            bth_sb = spool.tile([N, N], F32)
            nc.vector.tensor_sub(out=bth_sb[:], in0=s_ps[:], in1=skew_sb[:])

            # ---- X0 = Bh^T / ||Bh||_F^2 (guaranteed Newton-Schulz contraction) ----
            sq_sb = spool.tile([N, N], F32)
            nc.vector.tensor_mul(out=sq_sb[:], in0=bh_sb[:], in1=bh_sb[:])
            rs_sb = spool.tile([N, 1], F32)
            nc.vector.reduce_sum(rs_sb[:], sq_sb[:], axis=mybir.AxisListType.X)
            # ones64^T @ rs: reduces over partitions AND broadcasts the total
            # to all 64 partitions in a single matmul.
            fro_ps = pps.tile([N, 1], F32, tag="pro")
            nc.tensor.matmul(fro_ps[:], lhsT=ones64[:], rhs=rs_sb[:], start=True, stop=True)
            rcp_sb = spool.tile([N, 1], F32)
            nc.vector.reciprocal(out=rcp_sb[:], in_=fro_ps[:])

            # ---- burn-in by repeated squaring ----
            # E = I - Bh Bh^T/s is symmetric with spectrum in (0,1); all its
            # powers commute, so X_K = (Bh^T/s) * W with
            # W = prod_{j<N_SQUARE}(I + E^(2^j)), symmetric.  The F-chain
            # (F <- F@F) is the only serial recurrence; W is assembled as a
            # pairwise product tree OFF the critical path, with the leaf
            # pairs (I+Fa)(I+Fb) = Fa@Fb + Fa + Fb + I done by pure PSUM
            # accumulation (no vector ops).
            p0_ps = nps.tile([N, N], F32, tag="nw")
            nc.tensor.matmul(p0_ps[:], lhsT=bth_sb[:], rhs=bth_sb[:], start=True, stop=True)
            p0s_sb = spool.tile([N, N], F32)
            nc.vector.tensor_scalar_mul(p0s_sb[:], p0_ps[:], rcp_sb[:])

            f0 = fpool.tile([N, N], F32, tag="f0", name="f0")
            nc.vector.tensor_sub(out=f0[:], in0=eye[:], in1=p0s_sb[:])

            f_list = [f0]
            for j in range(1, N_SQUARE):
                f2_ps = nps.tile([N, N], F32, tag="nw")
                nc.tensor.matmul(
                    f2_ps[:], lhsT=f_list[-1][:], rhs=f_list[-1][:],
                    start=True, stop=True,
                )
                fj = fpool.tile([N, N], F32, tag=f"f{j}", name=f"f{j}")
                nc.vector.tensor_copy(out=fj[:], in_=f2_ps[:])
                f_list.append(fj)

            # leaf pairs: (I+Fa)(I+Fb) via 4 accumulating matmuls
            nodes = []
            li = 0
            i = 0
            while i + 1 < len(f_list):
                fa, fb = f_list[i], f_list[i + 1]
                h_ps = pps.tile([N, N], F32, tag="pro")
                nc.tensor.matmul(h_ps[:], lhsT=fa[:], rhs=fb[:], start=True, stop=False)
                nc.tensor.matmul(h_ps[:], lhsT=eye[:], rhs=fa[:], start=False, stop=False)
                nc.tensor.matmul(h_ps[:], lhsT=eye[:], rhs=fb[:], start=False, stop=False)
                nc.tensor.matmul(h_ps[:], lhsT=eye[:], rhs=eye[:], start=False, stop=True)
                h_sb = fpool.tile([N, N], F32, tag=f"h{li}", name=f"h{li}")
                li += 1
                nc.scalar.copy(out=h_sb[:], in_=h_ps[:])
                nodes.append(h_sb)
                i += 2
            if i < len(f_list):
                g_sb = fpool.tile([N, N], F32, tag=f"h{li}", name=f"h{li}")
                li += 1
                nc.vector.tensor_add(out=g_sb[:], in0=eye[:], in1=f_list[i][:])
                nodes.append(g_sb)

            # binary product tree over the pair-leaves: within-level
            # parallelism keeps the post-chain tail short (the factors
            # commute, so any association is valid).
            while len(nodes) > 1:
                nxt = []
                for k in range(0, len(nodes) - 1, 2):
                    t_ps = pps.tile([N, N], F32, tag="pro")
                    nc.tensor.matmul(
                        t_ps[:], lhsT=nodes[k][:], rhs=nodes[k + 1][:],
                        start=True, stop=True,
                    )
                    t_sb = fpool.tile([N, N], F32, tag=f"h{li}", name=f"h{li}")
                    li += 1
                    nc.scalar.copy(out=t_sb[:], in_=t_ps[:])
                    nxt.append(t_sb)
                if len(nodes) % 2:
                    nxt.append(nodes[-1])
                nodes = nxt
            w_sb = nodes[0]

            # ---- X = (Bh^T W)/s,  X^T = (W Bh)/s ----
            xx_ps = nps.tile([N, N], F32, tag="nw")
            nc.tensor.matmul(xx_ps[:], lhsT=bh_sb[:], rhs=w_sb[:], start=True, stop=True)
            x_sb = xpool_s.tile([N, N], F32, tag="x")
            nc.vector.tensor_scalar_mul(x_sb[:], xx_ps[:], rcp_sb[:])
            xxt_ps = nps.tile([N, N], F32, tag="nw")
            nc.tensor.matmul(xxt_ps[:], lhsT=w_sb[:], rhs=bh_sb[:], start=True, stop=True)
            xt_sb = xpool_s.tile([N, N], F32, tag="x")
            nc.vector.tensor_scalar_mul(xt_sb[:], xxt_ps[:], rcp_sb[:])

            # ---- Newton-Schulz polish: X <- X(2I - Bh X) (self-correcting) ----
            # (Q^T @ X^T only needs Q as lhsT, so Q^T is never materialized)
            for it in range(N_POLISH):
                last = it == N_POLISH - 1
                p_ps = nps.tile([N, N], F32, tag="nw")
                nc.tensor.matmul(p_ps[:], lhsT=bth_sb[:], rhs=x_sb[:], start=True, stop=True)

                q_sb = spool.tile([N, N], F32, tag="q")
                nc.vector.tensor_sub(out=q_sb[:], in0=two_i[:], in1=p_ps[:])

                xnt_ps = nps.tile([N, N], F32, tag="nw")
                nc.tensor.matmul(xnt_ps[:], lhsT=q_sb[:], rhs=xt_sb[:], start=True, stop=True)
                if not last:
                    xn_ps = nps.tile([N, N], F32, tag="nw")
                    nc.tensor.matmul(xn_ps[:], lhsT=xt_sb[:], rhs=q_sb[:], start=True, stop=True)
                    x_sb = xpool_s.tile([N, N], F32, tag="x")
                    nc.vector.tensor_copy(out=x_sb[:], in_=xn_ps[:])
                xt_sb = xpool_s.tile([N, N], F32, tag="x")
                nc.vector.tensor_copy(out=xt_sb[:], in_=xnt_ps[:])

            # ---- At = A^T = (M21/2)^T @ X^T  (exactly the lhsT the big matmul needs) ----
            at_ps = nps.tile([N, N], F32, tag="nw")
            nc.tensor.matmul(at_ps[:], lhsT=m21h_sb[:], rhs=xt_sb[:], start=True, stop=True)

            # ---- block-diagonal [[At, 0], [0, At]] in SBUF ----
            # (DVE memset + two DMAs from PSUM keeps the writer set small:
            #  only two distinct semaphores for the consuming matmuls)
            # f32r: the DVE copy out of PSUM rounds At to the replicated-fp32
            # format the PE needs; the moving operand then streams 1 row/cycle.
            at128 = cpool.tile([128, 128], F32R)
            nc.sync.dma_start(out=at128[:], in_=zero_ext[:, :])
            nc.vector.tensor_copy(out=at128[0:N, 0:N], in_=at_ps[:])
            # cross-partition move for the lower block: SBUF->SBUF DMA
            nc.sync.dma_start(out=at128[N:128, N:128], in_=at128[0:N, 0:N])

            # ---- streaming matmul: out = blockdiag(At)^T @ x_shard ----
            for c in range(t_half // OUT_COLS):
                obase, ocols = c * OUT_COLS, OUT_COLS
                yout_full = opool.tile([128, OUT_COLS], F32, tag="yout", name="yout")
                yout = yout_full[:]
                for h in range(ocols // DMA_COLS):
                    xin = xpool.tile([128, DMA_COLS], F32R, tag="xin")
                    base = obase + h * DMA_COLS
                    nc.sync.dma_start(
                        out=xin[:], in_=x_ext[:, base : base + DMA_COLS]
                    )
                    for j in range(DMA_COLS // MM_COLS):
                        ps = mps.tile([128, MM_COLS], F32, tag="mm")
                        nc.tensor.matmul(
                            ps[:],
                            lhsT=at128[:],
                            rhs=xin[:, j * MM_COLS : (j + 1) * MM_COLS],
                            start=True,
                            stop=True,
                        )
                        dst = yout[:, h * DMA_COLS + j * MM_COLS
                                   : h * DMA_COLS + (j + 1) * MM_COLS]
                        if j % 3 == 2:
                            nc.scalar.copy(out=dst, in_=ps[:])
                        else:
                            nc.vector.tensor_copy(out=dst, in_=ps[:])
                nc.sync.dma_start(
                    out=out_ext[:, obase : obase + ocols], in_=yout[:]
                )

    return nc


_NC_CACHE = {}
LAST_PROFILE = None


def _get_nc(t_half=T_HALF):
    if t_half not in _NC_CACHE:
        nc = build_kernel(t_half)
        nc.finalize()  # Bacc: reg alloc + event-semaphore wait splitting
        _NC_CACHE[t_half] = nc
    return _NC_CACHE[t_half]


def _ensure_ntff_hook():
    """The agent image's `antenv` lacks the `axon_hooks` shim that
    `trn_agent_boot` uses to register the NTFF profiling hook (boot
    degrades silently).  Provide the shim and register the hook so
    run_bass_kernel_spmd(trace=True) can capture neuron-profile data."""
    import types

    try:
        from antenv.axon_hooks import get_axon_ntff_profile_hook  # noqa: F401
        return True
    except ImportError:
        pass
    try:
        import antenv
        from trn_agent_boot.trn_boot import _ntff_profile_via_ctypes

        mod = types.ModuleType("antenv.axon_hooks")
        _store = {"h": None}
        mod.set_axon_ntff_profile_hook = lambda h: _store.__setitem__("h", h)
        mod.get_axon_ntff_profile_hook = lambda: _store["h"]
        sys.modules["antenv.axon_hooks"] = mod
        antenv.axon_hooks = mod
        hook = _ntff_profile_via_ctypes("/opt/axon/libaxon_pjrt.so")
        mod.set_axon_ntff_profile_hook(hook)
        return hook is not None
    except Exception as e:  # degrade to no-trace
        print(f"kernel.py: NTFF hook setup failed ({type(e).__name__}: {e})")
        return False


def kernel(x, L, R):
    global LAST_PROFILE
    x = np.ascontiguousarray(np.asarray(x, dtype=np.float32))
    L = np.ascontiguousarray(np.asarray(L, dtype=np.float32))
    R = np.ascontiguousarray(np.asarray(R, dtype=np.float32))
    assert x.shape == (T_FULL, N), x.shape

    X = x.reshape(N, T_FULL)  # row-major view, no copy
    Lt = np.ascontiguousarray(L.T)
    Rt = np.ascontiguousarray(R.T)
    twoi = (2.0 * np.eye(N)).astype(np.float32)
    eyen = np.eye(N, dtype=np.float32)
    zero128 = np.zeros((128, 128), dtype=np.float32)

    in_maps = []
    for c in range(N_CORES):
        shard = np.empty((128, T_HALF), dtype=np.float32)
        base = c * T_CORE
        shard[:N] = X[:, base : base + T_HALF]
        shard[N:] = X[:, base + T_HALF : base + T_CORE]
        in_maps.append({"x": shard, "Lt": Lt, "R": R, "Rt": Rt,
                        "TWOI": twoi, "EYE": eyen, "ZERO128": zero128})

    nc = _get_nc()
    trace = os.environ.get("KERNEL_TRACE", "0") == "1"
    if trace:
        trace = _ensure_ntff_hook()
    try:
        res = run_bass_kernel_spmd(
            nc, in_maps, core_ids=list(range(N_CORES)), trace=trace
        )
    except Exception:
        if not trace:
            raise
        print("kernel.py: traced run failed; retrying without trace")
        res = run_bass_kernel_spmd(
            nc, in_maps, core_ids=list(range(N_CORES)), trace=False
        )
    LAST_PROFILE = res

    Y = np.empty((N, T_FULL), dtype=np.float32)
    for c in range(N_CORES):
        o = res.results[c]["out"]
        base = c * T_CORE
        Y[:, base : base + T_HALF] = o[:N]
        Y[:, base + T_HALF : base + T_CORE] = o[N:]
    return Y.reshape(T_FULL, N)



# revision 5
# speedup vs baseline: 1.7068x; 1.7068x over previous
"""Distributed Trainium2 kernel for the Koopman-operator problem.

Math (from the reference):
    X  = x.reshape(64, T)                 # T = 524288, pure row-major view
    M  = L @ L.T                          # 128x128;  M11, M21, M22 are 64x64 blocks
    B  = 2*(M11 + M22 + R - R.T)          # (eps*I is ~1e-8, negligible vs O(30) entries)
    A  = inv(B) @ M21
    out = (A @ X).reshape(-1, 64)

Distribution: column-shard X across 8 cores (65536 cols each) -- fully
data-parallel, zero collectives.  L and R are replicated; every core
redundantly computes A on device via a Frobenius-scaled Newton-Schulz
iteration (B is well conditioned, cond ~650 -> ~21 squarings + 1 polish
converge to ~1e-4, far inside the 2e-2 gate).

I/O is fp16: the host casts the X shard to fp16 (quantization ~1.5e-4
rel) and upcasts the fp16 result; this halves HBM traffic, which is
the roofline for this kernel.  Per core the shard is pre-stacked on
host as (128, 32768): rows 0:64 hold the first 32768 columns, rows
64:128 the next 32768.  The stationary matrix for the big matmul is
the block-diagonal [[A^T, 0], [0, A^T]] (128x128 fp16), which doubles
PE utilization (K=128, M=128 instead of 64).

Stream-phase structure (the part that sets the wall clock):
  - all 8 input DMAs are issued up front (whole fp16 shard is SBUF
    resident; 64 KiB/partition) so they overlap the Newton-Schulz chain
  - 5 PSUM banks rotate under the 64 streaming matmuls so the PE never
    stalls on a bank and stays ramped at full clock
  - PSUM->SBUF fp16 cast copies round-robin over vector/scalar/gpsimd
  - output DMAs go out every 2048 columns from the sync engine
"""

import os
import sys

import numpy as np

for _p in ("/opt/trn_rl_repo", "/root/.axon_site/_ro/trn_rl_repo"):
    if _p not in sys.path and os.path.isdir(_p):
        sys.path.append(_p)

import concourse.bass as bass
import concourse.mybir as mybir
from concourse import bacc
from concourse.bass_utils import run_bass_kernel_spmd

from concourse.tile import TileContext

F32 = mybir.dt.float32
F16 = mybir.dt.float16

N = 64                   # state dim
N_CORES = 8
T_FULL = 524288          # columns of the reshaped X
T_CORE = T_FULL // N_CORES       # 65536 columns per core
T_HALF = T_CORE // 2             # 32768 -> free dim of the (128, .) shard

N_SQUARE = 21            # number of product factors (I + E^(2^j)), j=0..20
N_POLISH = 1             # self-correcting Newton polish iterations
MM_COLS = 512            # matmul moving free dim (one PSUM bank, f32)
DMA_COLS = 4096          # input DMA chunk = 128 x 4096 x 2B = 1 MiB
OUT_COLS = 2048          # output DMA chunk = 0.5 MiB


def build_kernel(t_half=T_HALF):
    nc = bacc.Bacc()

    x_ext = nc.declare_dram_parameter("x", [128, t_half], F16, isOutput=False)
    lt_ext = nc.declare_dram_parameter("Lt", [2 * N, 2 * N], F32, isOutput=False)
    r_ext = nc.declare_dram_parameter("R", [N, N], F32, isOutput=False)
    rt_ext = nc.declare_dram_parameter("Rt", [N, N], F32, isOutput=False)
    twoi_ext = nc.declare_dram_parameter("TWOI", [N, N], F32, isOutput=False)
    eye_ext = nc.declare_dram_parameter("EYE", [N, N], F32, isOutput=False)
    out_ext = nc.declare_dram_parameter("out", [128, t_half], F16, isOutput=True)

    n_chunks = t_half // DMA_COLS

    with TileContext(nc) as tc:
        with (
            tc.tile_pool(name="const", bufs=1) as cpool,
            tc.tile_pool(name="small", bufs=2) as spool,
            tc.tile_pool(name="fch", bufs=1) as fpool,
            tc.tile_pool(name="xstate", bufs=2) as xpool_s,
            tc.tile_pool(name="xin", bufs=1) as xpool,
            tc.tile_pool(name="yout", bufs=4) as opool,
            tc.tile_pool(name="pro_ps", bufs=2, space="PSUM") as pps,
            tc.tile_pool(name="nw_ps", bufs=1, space="PSUM") as nps,
            tc.tile_pool(name="mm_ps", bufs=5, space="PSUM") as mps,
        ):
            # ---- load params (Lt/Rt are host-side layout of replicated L/R) ----
            lt_sb = spool.tile([128, 128], F32)
            nc.sync.dma_start(out=lt_sb[:], in_=lt_ext[:, :])
            r_sb = spool.tile([N, N], F32)
            nc.sync.dma_start(out=r_sb[:], in_=r_ext[:, :])
            rt_sb = spool.tile([N, N], F32)
            nc.sync.dma_start(out=rt_sb[:], in_=rt_ext[:, :])
            two_i = cpool.tile([N, N], F32)
            nc.sync.dma_start(out=two_i[:], in_=twoi_ext[:, :])
            eye = cpool.tile([N, N], F32)
            nc.sync.dma_start(out=eye[:], in_=eye_ext[:, :])

            # ---- whole fp16 input shard -> SBUF, issued before any compute ----
            # (overlaps the entire Newton-Schulz phase; 64 KiB/partition)
            xin = []
            for h in range(n_chunks):
                xt = xpool.tile([128, DMA_COLS], F16, tag=f"xin{h}", name=f"xin{h}")
                nc.sync.dma_start(
                    out=xt[:], in_=x_ext[:, h * DMA_COLS : (h + 1) * DMA_COLS]
                )
                xin.append(xt)

            # ---- constants (DVE memsets; cheap) ----
            ones64 = cpool.tile([N, N], F32)
            nc.vector.memset(ones64[:], 1.0)

            # ---- S = M11 + M22 = L1@L1^T + L2@L2^T  (PSUM accumulation) ----
            s_ps = pps.tile([N, N], F32, tag="pro")
            nc.tensor.matmul(
                s_ps[:], lhsT=lt_sb[:, 0:N], rhs=lt_sb[:, 0:N], start=True, stop=False
            )
            nc.tensor.matmul(
                s_ps[:], lhsT=lt_sb[:, N:128], rhs=lt_sb[:, N:128],
                start=False, stop=True,
            )

            # ---- M21h = 0.5 * M21 = 0.5 * L1 @ L2^T ----
            # (the 0.5 folds away the factor 2 in B: inv(2*Bh) @ M21 = inv(Bh) @ (M21/2))
            m21_ps = pps.tile([N, N], F32, tag="pro")
            nc.tensor.matmul(
                m21_ps[:], lhsT=lt_sb[:, 0:N], rhs=lt_sb[:, N:128],
                start=True, stop=True,
            )
            m21h_sb = spool.tile([N, N], F32)
            nc.vector.tensor_scalar_mul(m21h_sb[:], m21_ps[:], 0.5)

            # ---- skew = R - R^T ----
            skew_sb = spool.tile([N, N], F32)
            nc.vector.tensor_sub(out=skew_sb[:], in0=r_sb[:], in1=rt_sb[:])

            # ---- Bh = S + skew (= B/2), Bth = S - skew (= B^T/2) ----
            bh_sb = spool.tile([N, N], F32)
            nc.vector.tensor_add(out=bh_sb[:], in0=s_ps[:], in1=skew_sb[:])
            bth_sb = spool.tile([N, N], F32)
            nc.vector.tensor_sub(out=bth_sb[:], in0=s_ps[:], in1=skew_sb[:])

            # ---- X0 = Bh^T / ||Bh||_F^2 (guaranteed Newton-Schulz contraction) ----
            sq_sb = spool.tile([N, N], F32)
            nc.vector.tensor_mul(out=sq_sb[:], in0=bh_sb[:], in1=bh_sb[:])
            rs_sb = spool.tile([N, 1], F32)
            nc.vector.reduce_sum(rs_sb[:], sq_sb[:], axis=mybir.AxisListType.X)
            # ones64^T @ rs: reduces over partitions AND broadcasts the total
            # to all 64 partitions in a single matmul.
            fro_ps = pps.tile([N, 1], F32, tag="pro")
            nc.tensor.matmul(fro_ps[:], lhsT=ones64[:], rhs=rs_sb[:], start=True, stop=True)
            rcp_sb = spool.tile([N, 1], F32)
            nc.vector.reciprocal(out=rcp_sb[:], in_=fro_ps[:])

            # ---- burn-in by repeated squaring ----
            # E = I - Bh Bh^T/s is symmetric with spectrum in (0,1); all its
            # powers commute, so X_K = (Bh^T/s) * W with
            # W = prod_{j<N_SQUARE}(I + E^(2^j)), symmetric.  The F-chain
            # (F <- F@F) is the only serial recurrence; W is assembled as a
            # pairwise product tree OFF the critical path, with the leaf
            # pairs (I+Fa)(I+Fb) = Fa@Fb + Fa + Fb + I done by pure PSUM
            # accumulation (no vector ops).
            p0_ps = nps.tile([N, N], F32, tag="nw")
            nc.tensor.matmul(p0_ps[:], lhsT=bth_sb[:], rhs=bth_sb[:], start=True, stop=True)
            p0s_sb = spool.tile([N, N], F32)
            nc.vector.tensor_scalar_mul(p0s_sb[:], p0_ps[:], rcp_sb[:])

            f0 = fpool.tile([N, N], F32, tag="f0", name="f0")
            nc.vector.tensor_sub(out=f0[:], in0=eye[:], in1=p0s_sb[:])

            f_list = [f0]
            for j in range(1, N_SQUARE):
                f2_ps = nps.tile([N, N], F32, tag="nw")
                nc.tensor.matmul(
                    f2_ps[:], lhsT=f_list[-1][:], rhs=f_list[-1][:],
                    start=True, stop=True,
                )
                fj = fpool.tile([N, N], F32, tag=f"f{j}", name=f"f{j}")
                nc.vector.tensor_copy(out=fj[:], in_=f2_ps[:])
                f_list.append(fj)

            # leaf pairs: (I+Fa)(I+Fb) via 4 accumulating matmuls
            nodes = []
            li = 0
            i = 0
            while i + 1 < len(f_list):
                fa, fb = f_list[i], f_list[i + 1]
                h_ps = pps.tile([N, N], F32, tag="pro")
                nc.tensor.matmul(h_ps[:], lhsT=fa[:], rhs=fb[:], start=True, stop=False)
                nc.tensor.matmul(h_ps[:], lhsT=eye[:], rhs=fa[:], start=False, stop=False)
                nc.tensor.matmul(h_ps[:], lhsT=eye[:], rhs=fb[:], start=False, stop=False)
                nc.tensor.matmul(h_ps[:], lhsT=eye[:], rhs=eye[:], start=False, stop=True)
                h_sb = fpool.tile([N, N], F32, tag=f"h{li}", name=f"h{li}")
                li += 1
                nc.scalar.copy(out=h_sb[:], in_=h_ps[:])
                nodes.append(h_sb)
                i += 2
            if i < len(f_list):
                g_sb = fpool.tile([N, N], F32, tag=f"h{li}", name=f"h{li}")
                li += 1
                nc.vector.tensor_add(out=g_sb[:], in0=eye[:], in1=f_list[i][:])
                nodes.append(g_sb)

            # binary product tree over the pair-leaves: within-level
            # parallelism keeps the post-chain tail short (the factors
            # commute, so any association is valid).
            while len(nodes) > 1:
                nxt = []
                for k in range(0, len(nodes) - 1, 2):
                    t_ps = pps.tile([N, N], F32, tag="pro")
                    nc.tensor.matmul(
                        t_ps[:], lhsT=nodes[k][:], rhs=nodes[k + 1][:],
                        start=True, stop=True,
                    )
                    t_sb = fpool.tile([N, N], F32, tag=f"h{li}", name=f"h{li}")
                    li += 1
                    nc.scalar.copy(out=t_sb[:], in_=t_ps[:])
                    nxt.append(t_sb)
                if len(nodes) % 2:
                    nxt.append(nodes[-1])
                nodes = nxt
            w_sb = nodes[0]

            # ---- X = (Bh^T W)/s,  X^T = (W Bh)/s ----
            xx_ps = nps.tile([N, N], F32, tag="nw")
            nc.tensor.matmul(xx_ps[:], lhsT=bh_sb[:], rhs=w_sb[:], start=True, stop=True)
            x_sb = xpool_s.tile([N, N], F32, tag="x")
            nc.vector.tensor_scalar_mul(x_sb[:], xx_ps[:], rcp_sb[:])
            xxt_ps = nps.tile([N, N], F32, tag="nw")
            nc.tensor.matmul(xxt_ps[:], lhsT=w_sb[:], rhs=bh_sb[:], start=True, stop=True)
            xt_sb = xpool_s.tile([N, N], F32, tag="x")
            nc.vector.tensor_scalar_mul(xt_sb[:], xxt_ps[:], rcp_sb[:])

            # ---- Newton-Schulz polish: X <- X(2I - Bh X) (self-correcting) ----
            # (Q^T @ X^T only needs Q as lhsT, so Q^T is never materialized)
            for it in range(N_POLISH):
                last = it == N_POLISH - 1
                p_ps = nps.tile([N, N], F32, tag="nw")
                nc.tensor.matmul(p_ps[:], lhsT=bth_sb[:], rhs=x_sb[:], start=True, stop=True)

                q_sb = spool.tile([N, N], F32, tag="q")
                nc.vector.tensor_sub(out=q_sb[:], in0=two_i[:], in1=p_ps[:])

                xnt_ps = nps.tile([N, N], F32, tag="nw")
                nc.tensor.matmul(xnt_ps[:], lhsT=q_sb[:], rhs=xt_sb[:], start=True, stop=True)
                if not last:
                    xn_ps = nps.tile([N, N], F32, tag="nw")
                    nc.tensor.matmul(xn_ps[:], lhsT=xt_sb[:], rhs=q_sb[:], start=True, stop=True)
                    x_sb = xpool_s.tile([N, N], F32, tag="x")
                    nc.vector.tensor_copy(out=x_sb[:], in_=xn_ps[:])
                xt_sb = xpool_s.tile([N, N], F32, tag="x")
                nc.vector.tensor_copy(out=xt_sb[:], in_=xnt_ps[:])

            # ---- At = A^T = (M21/2)^T @ X^T  (exactly the lhsT the big matmul needs) ----
            at_ps = nps.tile([N, N], F32, tag="nw")
            nc.tensor.matmul(at_ps[:], lhsT=m21h_sb[:], rhs=xt_sb[:], start=True, stop=True)

            # ---- block-diagonal [[At, 0], [0, At]] in fp16 SBUF ----
            # memset the zeros, cast-copy the top block from PSUM, then an
            # SBUF->SBUF DMA for the lower block (engines cannot move data
            # across partitions; DMA can).
            at128 = cpool.tile([128, 128], F16)
            nc.vector.memset(at128[:], 0.0)
            nc.vector.tensor_copy(out=at128[0:N, 0:N], in_=at_ps[:])
            nc.sync.dma_start(out=at128[N:128, N:128], in_=at128[0:N, 0:N])

            # ---- streaming matmul: out = blockdiag(At)^T @ x_shard ----
            copy_engines = (nc.vector.tensor_copy, nc.scalar.copy)
            g = 0
            for c in range(t_half // OUT_COLS):
                obase = c * OUT_COLS
                yout = opool.tile([128, OUT_COLS], F16, tag="yout", name="yout")
                for j in range(OUT_COLS // MM_COLS):
                    col = obase + j * MM_COLS
                    xt = xin[col // DMA_COLS]
                    off = col % DMA_COLS
                    ps = mps.tile([128, MM_COLS], F32, tag="mm")
                    nc.tensor.matmul(
                        ps[:],
                        lhsT=at128[:],
                        rhs=xt[:, off : off + MM_COLS],
                        start=True,
                        stop=True,
                    )
                    dst = yout[:, j * MM_COLS : (j + 1) * MM_COLS]
                    copy_engines[g % 2](out=dst, in_=ps[:])
                    g += 1
                nc.sync.dma_start(
                    out=out_ext[:, obase : obase + OUT_COLS], in_=yout[:]
                )

    return nc


_NC_CACHE = {}
LAST_PROFILE = None


def _get_nc(t_half=T_HALF):
    if t_half not in _NC_CACHE:
        nc = build_kernel(t_half)
        nc.finalize()  # Bacc: reg alloc + event-semaphore wait splitting
        _NC_CACHE[t_half] = nc
    return _NC_CACHE[t_half]


def _ensure_ntff_hook():
    """The agent image's `antenv` lacks the `axon_hooks` shim that
    `trn_agent_boot` uses to register the NTFF profiling hook (boot
    degrades silently).  Provide the shim and register the hook so
    run_bass_kernel_spmd(trace=True) can capture neuron-profile data."""
    import types

    try:
        from antenv.axon_hooks import get_axon_ntff_profile_hook  # noqa: F401
        return True
    except ImportError:
        pass
    try:
        import antenv
        from trn_agent_boot.trn_boot import _ntff_profile_via_ctypes

        mod = types.ModuleType("antenv.axon_hooks")
        _store = {"h": None}
        mod.set_axon_ntff_profile_hook = lambda h: _store.__setitem__("h", h)
        mod.get_axon_ntff_profile_hook = lambda: _store["h"]
        sys.modules["antenv.axon_hooks"] = mod
        antenv.axon_hooks = mod
        hook = _ntff_profile_via_ctypes("/opt/axon/libaxon_pjrt.so")
        mod.set_axon_ntff_profile_hook(hook)
        return hook is not None
    except Exception as e:  # degrade to no-trace
        print(f"kernel.py: NTFF hook setup failed ({type(e).__name__}: {e})")
        return False


def kernel(x, L, R):
    global LAST_PROFILE
    x = np.ascontiguousarray(np.asarray(x, dtype=np.float32))
    L = np.ascontiguousarray(np.asarray(L, dtype=np.float32))
    R = np.ascontiguousarray(np.asarray(R, dtype=np.float32))
    assert x.shape == (T_FULL, N), x.shape

    X = x.reshape(N, T_FULL)  # row-major view, no copy
    Lt = np.ascontiguousarray(L.T)
    Rt = np.ascontiguousarray(R.T)
    twoi = (2.0 * np.eye(N)).astype(np.float32)
    eyen = np.eye(N, dtype=np.float32)

    in_maps = []
    for c in range(N_CORES):
        shard = np.empty((128, T_HALF), dtype=np.float16)
        base = c * T_CORE
        shard[:N] = X[:, base : base + T_HALF]
        shard[N:] = X[:, base + T_HALF : base + T_CORE]
        in_maps.append({"x": shard, "Lt": Lt, "R": R, "Rt": Rt,
                        "TWOI": twoi, "EYE": eyen})

    nc = _get_nc()
    trace = os.environ.get("KERNEL_TRACE", "0") == "1"
    if trace:
        trace = _ensure_ntff_hook()
    try:
        res = run_bass_kernel_spmd(
            nc, in_maps, core_ids=list(range(N_CORES)), trace=trace
        )
    except Exception:
        if not trace:
            raise
        print("kernel.py: traced run failed; retrying without trace")
        res = run_bass_kernel_spmd(
            nc, in_maps, core_ids=list(range(N_CORES)), trace=False
        )
    LAST_PROFILE = res

    Y = np.empty((N, T_FULL), dtype=np.float32)
    for c in range(N_CORES):
        o = res.results[c]["out"]
        base = c * T_CORE
        Y[:, base : base + T_HALF] = o[:N]
        Y[:, base + T_HALF : base + T_CORE] = o[N:]
    return Y.reshape(T_FULL, N)


# revision 19
# speedup vs baseline: 1.8180x; 1.0651x over previous
"""Distributed Trainium2 kernel for the Koopman-operator problem.

Math (from the reference):
    X  = x.reshape(64, T)                 # T = 524288, pure row-major view
    M  = L @ L.T                          # 128x128;  M11, M21, M22 are 64x64 blocks
    B  = 2*(M11 + M22 + R - R.T)          # (eps*I is ~1e-8, negligible vs O(30) entries)
    A  = inv(B) @ M21
    out = (A @ X).reshape(-1, 64)

Distribution: column-shard X across 8 cores (65536 cols each) -- fully
data-parallel, zero collectives.  L and R are replicated; every core
redundantly computes inv(B/2) on device.

Inverse: Frobenius-scaled Newton-Schulz by repeated squaring.
E = I - Bh Bh^T/s (s = ||Bh||_F^2) is symmetric with spectrum in
(0,1); inv(Bh) = (Bh^T/s) prod_j (I + E^(2^j)).  The F-chain
(F <- F@F, 21 steps) is the only serial recurrence; the (I + F_j)
factors all have spectrum in [1,2] (well conditioned -- this is why
this form is robust to hardware matmul rounding where the faster
Chebyshev-scaled Newton variant, whose intermediate factors pass
near singularity, is not), and are multiplied in an incremental
pairwise tree interleaved with the chain.  One fp32 Newton polish
squares the residual; measured end-to-end error ~4e-4 vs 2e-2.

I/O is fp16 (host casts, device computes in fp16->f32 PSUM): halves
HBM traffic, which is one of the three balanced rooflines here.  Per
core the shard is pre-stacked on host as (128, 32768): rows 0:64 hold
the first 32768 columns, rows 64:128 the next.  The stationary matrix
of the streaming matmul is blockdiag(At, At) (128x128 fp16), doubling
PE utilization.  At is produced directly in both partition halves by
running the final small matmul twice with different output partition
offsets (engines cannot copy across partitions; this avoids an
SBUF->SBUF DMA on the critical path).

Stream phase: whole fp16 shard is SBUF-resident (DMAs issued before
any compute, overlapping the inverse); the chain PSUM pools are
released and the stream takes all 8 PSUM banks as two (128,2048)
tiles; each is drained by a single cast-copy alternating between the
vector and scalar engines (the copy engines are the stream-phase
bottleneck at ~17 us); output DMAs go out every 2048 columns.
"""

import os
import sys

import numpy as np

for _p in ("/opt/trn_rl_repo", "/root/.axon_site/_ro/trn_rl_repo"):
    if _p not in sys.path and os.path.isdir(_p):
        sys.path.append(_p)

import concourse.bass as bass
import concourse.mybir as mybir
from concourse import bacc
from concourse.bass_utils import run_bass_kernel_spmd

from concourse.tile import TileContext

F32 = mybir.dt.float32
F16 = mybir.dt.float16

N = 64                   # state dim
N_CORES = 8
T_FULL = 524288          # columns of the reshaped X
T_CORE = T_FULL // N_CORES       # 65536 columns per core
T_HALF = T_CORE // 2             # 32768 -> free dim of the (128, .) shard

N_SQUARE = 21            # factors (I + E^(2^j)), j=0..20

MM_COLS = 512            # matmul moving free dim (one PSUM bank, f32)
DMA_COLS = 4096          # input DMA chunk = 128 x 4096 x 2B = 1 MiB
OUT_COLS = 2048          # stream tile = 4 PSUM banks; output DMA = 0.5 MiB


def build_kernel(t_half=T_HALF):
    nc = bacc.Bacc()

    x_ext = nc.declare_dram_parameter("x", [128, t_half], F16, isOutput=False)
    lt_ext = nc.declare_dram_parameter("Lt", [2 * N, 2 * N], F32, isOutput=False)
    r_ext = nc.declare_dram_parameter("R", [N, N], F32, isOutput=False)
    rt_ext = nc.declare_dram_parameter("Rt", [N, N], F32, isOutput=False)
    twoi_ext = nc.declare_dram_parameter("TWOI", [N, N], F32, isOutput=False)
    eye_ext = nc.declare_dram_parameter("EYE", [N, N], F32, isOutput=False)
    out_ext = nc.declare_dram_parameter("out", [128, t_half], F16, isOutput=True)

    n_chunks = t_half // DMA_COLS

    with TileContext(nc) as tc:
        with (
            tc.tile_pool(name="const", bufs=1) as cpool,
            tc.tile_pool(name="small", bufs=2) as spool,
            tc.tile_pool(name="uch", bufs=1) as upool,
            tc.tile_pool(name="xin", bufs=1) as xpool,
            tc.tile_pool(name="yout", bufs=4) as opool,
        ):
            # ---- load params ----
            lt_sb = spool.tile([128, 128], F32)
            nc.sync.dma_start(out=lt_sb[:], in_=lt_ext[:, :])
            r_sb = spool.tile([N, N], F32)
            nc.sync.dma_start(out=r_sb[:], in_=r_ext[:, :])
            rt_sb = spool.tile([N, N], F32)
            nc.sync.dma_start(out=rt_sb[:], in_=rt_ext[:, :])
            two_i = cpool.tile([N, N], F32)
            nc.sync.dma_start(out=two_i[:], in_=twoi_ext[:, :])
            eye = cpool.tile([N, N], F32)
            nc.sync.dma_start(out=eye[:], in_=eye_ext[:, :])

            # ---- whole fp16 input shard -> SBUF, issued before any compute ----
            xin = []
            for h in range(n_chunks):
                xt = xpool.tile([128, DMA_COLS], F16, tag=f"xin{h}", name=f"xin{h}")
                nc.sync.dma_start(
                    out=xt[:], in_=x_ext[:, h * DMA_COLS : (h + 1) * DMA_COLS]
                )
                xin.append(xt)

            # ---- cheap constants (off the critical path) ----
            ones64 = cpool.tile([N, N], F32)
            nc.vector.memset(ones64[:], 1.0)
            at128 = cpool.tile([128, 128], F16)
            nc.vector.memset(at128[:], 0.0)

            with (
                tc.tile_pool(name="pro_ps", bufs=3, space="PSUM") as pps,
                tc.tile_pool(name="nw_ps", bufs=2, space="PSUM") as nps,
            ):
                # ---- S = M11 + M22 = L1@L1^T + L2@L2^T  (PSUM accumulation) ----
                s_ps = pps.tile([N, N], F32, tag="pro")
                nc.tensor.matmul(
                    s_ps[:], lhsT=lt_sb[:, 0:N], rhs=lt_sb[:, 0:N],
                    start=True, stop=False,
                )
                nc.tensor.matmul(
                    s_ps[:], lhsT=lt_sb[:, N:128], rhs=lt_sb[:, N:128],
                    start=False, stop=True,
                )

                # ---- M21 = L1 @ L2^T (the 1/2 of M21h folds into the At copy) ----
                m21_ps = pps.tile([N, N], F32, tag="pro")
                nc.tensor.matmul(
                    m21_ps[:], lhsT=lt_sb[:, 0:N], rhs=lt_sb[:, N:128],
                    start=True, stop=True,
                )

                # ---- skew = R - R^T;  Bh = S + skew;  Bth = S - skew = Bh^T ----
                skew_sb = spool.tile([N, N], F32)
                nc.vector.tensor_sub(out=skew_sb[:], in0=r_sb[:], in1=rt_sb[:])
                bh_sb = spool.tile([N, N], F32)
                nc.vector.tensor_add(out=bh_sb[:], in0=s_ps[:], in1=skew_sb[:])
                bth_sb = spool.tile([N, N], F32)
                nc.vector.tensor_sub(out=bth_sb[:], in0=s_ps[:], in1=skew_sb[:])

                # ---- s = ||Bh||_F^2: ACT square w/ row-accumulate, then a
                # ones-matmul to reduce over partitions + broadcast ----
                sq_sb = spool.tile([N, N], F32)
                nc.vector.tensor_mul(out=sq_sb[:], in0=bh_sb[:], in1=bh_sb[:])
                rs_sb = spool.tile([N, 1], F32)
                nc.vector.reduce_sum(rs_sb[:], sq_sb[:], axis=mybir.AxisListType.X)
                fro_ps = pps.tile([N, 1], F32, tag="pro")
                nc.tensor.matmul(fro_ps[:], lhsT=ones64[:], rhs=rs_sb[:],
                                 start=True, stop=True)
                rcp_sb = spool.tile([N, 1], F32)
                nc.vector.reciprocal(out=rcp_sb[:], in_=fro_ps[:])

                # m21 out of PSUM (off the critical path, ACT engine)
                m21_sb = spool.tile([N, N], F32)
                nc.scalar.copy(out=m21_sb[:], in_=m21_ps[:])

                # ---- F0 = E = I - Bh Bh^T / s ----
                p0_ps = nps.tile([N, N], F32, tag="nw")
                nc.tensor.matmul(p0_ps[:], lhsT=bth_sb[:], rhs=bth_sb[:],
                                 start=True, stop=True)
                p0s_sb = spool.tile([N, N], F32)
                nc.vector.tensor_scalar_mul(p0s_sb[:], p0_ps[:], rcp_sb[:])
                f_prev = upool.tile([N, N], F32, tag="f0", name="f0")
                nc.vector.tensor_sub(out=f_prev[:], in0=eye[:], in1=p0s_sb[:])

                # ---- F-chain F <- F@F with the G_j = I + F_j factors and
                # their product tree interleaved off the critical path ----
                tree_q = []        # ready product nodes awaiting pairing
                li = 0

                def emit_g(f_tile, j):
                    g = upool.tile([N, N], F32, tag=f"g{j}", name=f"g{j}")
                    nc.vector.tensor_add(out=g[:], in0=eye[:], in1=f_tile[:])
                    tree_q.append(g)

                def emit_tree_mm(budget):
                    nonlocal li
                    done = 0
                    while done < budget and len(tree_q) >= 2:
                        a = tree_q.pop(0)
                        b = tree_q.pop(0)
                        t_ps = pps.tile([N, N], F32, tag="pro")
                        nc.tensor.matmul(t_ps[:], lhsT=a[:], rhs=b[:],
                                         start=True, stop=True)
                        t_sb = upool.tile([N, N], F32, tag=f"h{li}",
                                          name=f"h{li}")
                        nc.scalar.copy(out=t_sb[:], in_=t_ps[:])
                        li += 1
                        tree_q.append(t_sb)
                        done += 1

                emit_g(f_prev, 0)
                for j in range(1, N_SQUARE):
                    f2_ps = nps.tile([N, N], F32, tag="nw")
                    nc.tensor.matmul(f2_ps[:], lhsT=f_prev[:], rhs=f_prev[:],
                                     start=True, stop=True)
                    f_j = upool.tile([N, N], F32, tag=f"f{j}", name=f"f{j}")
                    nc.vector.tensor_copy(out=f_j[:], in_=f2_ps[:])
                    if j < N_SQUARE - 1:
                        emit_g(f_j, j)
                        emit_tree_mm(1)
                    f_prev = f_j
                # product of G_0..G_19, then W = that @ G_20
                emit_tree_mm(N_SQUARE)
                assert len(tree_q) == 1
                w_pre = tree_q.pop()
                glast = upool.tile([N, N], F32, tag="glast", name="glast")
                nc.vector.tensor_add(out=glast[:], in0=eye[:], in1=f_prev[:])
                g_last_ps = pps.tile([N, N], F32, tag="pro")
                nc.tensor.matmul(g_last_ps[:], lhsT=w_pre[:], rhs=glast[:],
                                 start=True, stop=True)
                w_sb = upool.tile([N, N], F32, tag="w", name="w")
                nc.vector.tensor_copy(out=w_sb[:], in_=g_last_ps[:])

                # ---- X = (Bh^T W)/s,  X^T = (W Bh)/s ----
                xx_ps = nps.tile([N, N], F32, tag="nw")
                nc.tensor.matmul(xx_ps[:], lhsT=bh_sb[:], rhs=w_sb[:],
                                 start=True, stop=True)
                xxt_ps = nps.tile([N, N], F32, tag="nw")
                nc.tensor.matmul(xxt_ps[:], lhsT=w_sb[:], rhs=bh_sb[:],
                                 start=True, stop=True)
                x_sb = upool.tile([N, N], F32, tag="x", name="x")
                nc.vector.tensor_scalar_mul(x_sb[:], xx_ps[:], rcp_sb[:])
                xt_sb = upool.tile([N, N], F32, tag="xt", name="xt")
                nc.vector.tensor_scalar_mul(xt_sb[:], xxt_ps[:], rcp_sb[:])

                # ---- Newton polish: X <- X(2I - Bh X) (self-correcting) ----
                p_ps = nps.tile([N, N], F32, tag="nw")
                nc.tensor.matmul(p_ps[:], lhsT=bth_sb[:], rhs=x_sb[:],
                                 start=True, stop=True)
                q_sb = spool.tile([N, N], F32, tag="q")
                nc.vector.tensor_sub(out=q_sb[:], in0=two_i[:], in1=p_ps[:])
                xnt_ps = nps.tile([N, N], F32, tag="nw")
                nc.tensor.matmul(xnt_ps[:], lhsT=q_sb[:], rhs=xt_sb[:],
                                 start=True, stop=True)
                # fold the 1/2 of M21h into the polished X^T so the At
                # copies below are plain (unscaled) copies
                xt2_sb = upool.tile([N, N], F32, tag="xt2", name="xt2")
                nc.vector.tensor_scalar_mul(xt2_sb[:], xnt_ps[:], 0.5)

                # ---- At = 0.5 * M21^T @ X^T, produced in BOTH partition
                # halves (two matmuls, different output partition offsets) so
                # the blockdiag build needs no cross-partition move ----
                at_psa = nps.tile([N, N], F32, tag="nw")
                nc.tensor.matmul(at_psa[:], lhsT=m21_sb[:], rhs=xt2_sb[:],
                                 start=True, stop=True)
                at_psb = nps.tile([128, N], F32, tag="nw")
                nc.tensor.matmul(at_psb[N:128, 0:N], lhsT=m21_sb[:],
                                 rhs=xt2_sb[:], start=True, stop=True)
                nc.vector.tensor_copy(out=at128[0:N, 0:N], in_=at_psa[:])
                nc.scalar.copy(out=at128[N:128, N:128], in_=at_psb[N:128, 0:N])

            # ---- streaming matmul: out = blockdiag(At)^T @ x_shard ----
            # chain PSUM pools are released; the stream gets all 8 banks as
            # two (128, 2048) tiles, each drained by one cast-copy (scalar
            # engine is 1.2 GHz vs vector 0.96, so it takes the odd extra).
            with tc.tile_pool(name="mm_ps", bufs=2, space="PSUM") as mps:
                n_tiles = t_half // OUT_COLS
                for i in range(n_tiles):
                    obase = i * OUT_COLS
                    ps = mps.tile([128, OUT_COLS], F32, tag="mm")
                    for j in range(OUT_COLS // MM_COLS):
                        col = obase + j * MM_COLS
                        xt = xin[col // DMA_COLS]
                        off = col % DMA_COLS
                        nc.tensor.matmul(
                            ps[:, j * MM_COLS : (j + 1) * MM_COLS],
                            lhsT=at128[:],
                            rhs=xt[:, off : off + MM_COLS],
                            start=True,
                            stop=True,
                        )
                    yout = opool.tile([128, OUT_COLS], F16, tag="yout",
                                      name="yout")
                    if i % 2 == 0 or i == n_tiles - 1:
                        nc.scalar.copy(out=yout[:], in_=ps[:])
                    else:
                        nc.vector.tensor_copy(out=yout[:], in_=ps[:])
                    nc.sync.dma_start(
                        out=out_ext[:, obase : obase + OUT_COLS], in_=yout[:]
                    )

    return nc


_NC_CACHE = {}
LAST_PROFILE = None


def _get_nc(t_half=T_HALF):
    if t_half not in _NC_CACHE:
        nc = build_kernel(t_half)
        nc.finalize()  # Bacc: reg alloc + event-semaphore wait splitting
        _NC_CACHE[t_half] = nc
    return _NC_CACHE[t_half]


def _ensure_ntff_hook():
    """The agent image's `antenv` lacks the `axon_hooks` shim that
    `trn_agent_boot` uses to register the NTFF profiling hook (boot
    degrades silently).  Provide the shim and register the hook so
    run_bass_kernel_spmd(trace=True) can capture neuron-profile data."""
    import types

    try:
        from antenv.axon_hooks import get_axon_ntff_profile_hook  # noqa: F401
        return True
    except ImportError:
        pass
    try:
        import antenv
        from trn_agent_boot.trn_boot import _ntff_profile_via_ctypes

        mod = types.ModuleType("antenv.axon_hooks")
        _store = {"h": None}
        mod.set_axon_ntff_profile_hook = lambda h: _store.__setitem__("h", h)
        mod.get_axon_ntff_profile_hook = lambda: _store["h"]
        sys.modules["antenv.axon_hooks"] = mod
        antenv.axon_hooks = mod
        hook = _ntff_profile_via_ctypes("/opt/axon/libaxon_pjrt.so")
        mod.set_axon_ntff_profile_hook(hook)
        return hook is not None
    except Exception as e:  # degrade to no-trace
        print(f"kernel.py: NTFF hook setup failed ({type(e).__name__}: {e})")
        return False


def kernel(x, L, R):
    global LAST_PROFILE
    x = np.ascontiguousarray(np.asarray(x, dtype=np.float32))
    L = np.ascontiguousarray(np.asarray(L, dtype=np.float32))
    R = np.ascontiguousarray(np.asarray(R, dtype=np.float32))
    assert x.shape == (T_FULL, N), x.shape

    X = x.reshape(N, T_FULL)  # row-major view, no copy
    Lt = np.ascontiguousarray(L.T)
    Rt = np.ascontiguousarray(R.T)
    twoi = (2.0 * np.eye(N)).astype(np.float32)
    eyen = np.eye(N, dtype=np.float32)

    in_maps = []
    for c in range(N_CORES):
        shard = np.empty((128, T_HALF), dtype=np.float16)
        base = c * T_CORE
        shard[:N] = X[:, base : base + T_HALF]
        shard[N:] = X[:, base + T_HALF : base + T_CORE]
        in_maps.append({"x": shard, "Lt": Lt, "R": R, "Rt": Rt,
                        "TWOI": twoi, "EYE": eyen})

    nc = _get_nc()
    trace = os.environ.get("KERNEL_TRACE", "0") == "1"
    if trace:
        trace = _ensure_ntff_hook()
    try:
        res = run_bass_kernel_spmd(
            nc, in_maps, core_ids=list(range(N_CORES)), trace=trace
        )
    except Exception:
        if not trace:
            raise
        print("kernel.py: traced run failed; retrying without trace")
        res = run_bass_kernel_spmd(
            nc, in_maps, core_ids=list(range(N_CORES)), trace=False
        )
    LAST_PROFILE = res

    Y = np.empty((N, T_FULL), dtype=np.float32)
    for c in range(N_CORES):
        o = res.results[c]["out"]
        base = c * T_CORE
        Y[:, base : base + T_HALF] = o[:N]
        Y[:, base + T_HALF : base + T_CORE] = o[N:]
    return Y.reshape(T_FULL, N)
